# revision 1
# baseline (speedup 1.0000x reference)
"""Causal multi-head attention (B=2, T=2048, C=1024, H=16) on 8 TRN2 cores.

Sharding: data-parallel over batch (2 groups of 4 cores), tensor-parallel
over heads within a group (4 heads / core). Each core:
  1. computes Q^T, K^T (layout [d, t]) and V (layout [t, d]) for its heads
     from x[b]^T (host-transposed) and its W column slices,
  2. runs causal attention in the S^T = K @ Q^T orientation (softmax sums
     come for free from a ones-column appended to V; softmax max-subtraction
     is skipped -- scores are O(1) here so exp is safe),
  3. computes a PARTIAL output projection over its 256 attention dims
     (contraction = its head slice of Wo rows) for the full [T, C] output,
  4. ReduceScatters the partials (op=add) within its 4-core batch group,
     which lands each core's T-shard of the final output directly in the
     rs_out ExternalOutput. Host reassembles/converts the shards.

Projection and attention emission is interleaved per 512-query chunk so the
first ReduceScatter (rows 0:1536) overlaps the last attention chunk; only the
second (rows 1536:2048) is tail-exposed. Matmul operands are bf16 (fp32 PSUM
accumulation); softmax normalization (reciprocal broadcast) stays in fp32.
"""

import os
import sys

import numpy as np
import ml_dtypes

for _p in ("/opt/trn_rl_repo",):
    if os.path.isdir(_p) and _p not in sys.path:
        sys.path.insert(0, _p)

import concourse.bacc as bacc
import concourse.mybir as mybir
import concourse.tile as tile
from concourse import bass_utils

B, T, C, H, D = 2, 2048, 1024, 16, 64
NCORES = 8
GP = 4              # cores per batch group
HPC = H // GP       # heads per core = 4
DS = HPC * D        # per-core head-dim slice = 256
NCT = C // 128      # c-tiles = 8
NQC = T // 512      # q-chunks = 4
NKT = T // 128      # k-tiles = 16
RSA_ROWS = 1536     # rows in the first ReduceScatter (qc 0..2)

F32 = mybir.dt.float32
F32R = mybir.dt.float32r
BF16 = mybir.dt.bfloat16
AF = mybir.ActivationFunctionType
ALU = mybir.AluOpType
NPBF = ml_dtypes.bfloat16

_PROG = None
LAST_RESULTS = None  # BassKernelResults of the most recent run (for test.py)


def _r(ap):
    return ap.bitcast(F32R)


def _normalize(nc, pn, psOut, ones_t, bv_sb, attn_sb, op_ps, h, qc, tag):
    """Divide O' rows 0..63 by the row-sums (row 64), add bv, write attn."""
    recip = pn.tile([65, 512], F32R, tag="recip")
    with nc.allow_low_precision(
        reason="f32r typing for matmul feed; storage is fp32"
    ):
        nc.vector.reciprocal(recip[64:65, :], op_ps[64:65, :])
    bc_ps = psOut.tile([64, 512], F32, tag=tag, name=f"bc_{h}_{qc}")
    nc.tensor.matmul(
        bc_ps[:, :],
        _r(ones_t[64:65, :]),
        _r(recip[64:65, :]),
        start=True,
        stop=True,
    )
    bc_sb = pn.tile([64, 512], F32, tag="bc")
    nc.vector.tensor_copy(bc_sb[:, :], bc_ps[:, :])
    po = 64 * (h % 2)
    aslc = attn_sb[(h // 2, qc // 2)][po : po + 64, 512 * (qc % 2) : 512 * (qc % 2 + 1)]
    nc.vector.tensor_tensor(aslc, op_ps[0:64, :], bc_sb[:, :], ALU.mult)
    nc.vector.tensor_scalar_add(aslc, aslc, bv_sb[:, h : h + 1])


def _emit(nc, tc, io):
    (xT, wq, wk, wv, wo2, bq2, bk2, bv4, bo4, maskd, onesd, onesv,
     partial, rs_tmp, rs_out) = io

    with (
        tc.tile_pool(name="outer", bufs=1) as po,
    ):
        # ---- small persistent tiles (scalar-engine DMA queue) ----------
        mask_sb = po.tile([128, 4 * 512], BF16, tag="mask")
        ones_t = po.tile([128, 64], F32R, tag="ones")
        nc.scalar.dma_start(ones_t[:, :], onesd[:, :].bitcast(F32R))
        bq_sb = po.tile([128, 2], F32, tag="bq")
        nc.scalar.dma_start(bq_sb[:, :], bq2[:, :])
        bk_sb = po.tile([128, 2], F32, tag="bk")
        nc.scalar.dma_start(bk_sb[:, :], bk2[:, :])
        bv_sb = po.tile([64, 4], F32, tag="bv")
        nc.scalar.dma_start(bv_sb[:, :], bv4[:, :])
        attn_sb = {}
        for hp in range(2):
            for hf in (0, 1):
                attn_sb[(hp, hf)] = po.tile(
                    [128, T // 2], BF16, tag=f"attn{hp}_{hf}", name=f"attn{hp}_{hf}"
                )

        with (
            tc.tile_pool(name="proj", bufs=1) as pp,
            tc.tile_pool(name="work", bufs=12) as pw,
            tc.tile_pool(name="nrm", bufs=2) as pn,
            tc.tile_pool(name="wos", bufs=1) as pwo,
            tc.tile_pool(name="osb", bufs=4) as posb,
            tc.tile_pool(name="fwd", bufs=2) as pfw,
            tc.tile_pool(name="psA", bufs=4, space="PSUM") as psA,
            tc.tile_pool(name="psO", bufs=1, space="PSUM") as psO,
            tc.tile_pool(name="psOut", bufs=1, space="PSUM") as psOut,
        ):
            # ---- input loads: xT split across the two HWDGE queues (SP
            # even tiles, Activation odd tiles); W triples ride SP after
            # their xT tile, bulky-but-later tensors trail on Activation -
            xT_sb = [None] * NCT
            for ci in range(1, NCT, 2):
                t_ = pp.tile([128, T], BF16, tag=f"xt{ci}", name=f"xt{ci}")
                nc.scalar.dma_start(t_[:, :], xT[128 * ci : 128 * (ci + 1), :])
                xT_sb[ci] = t_

            def _wtile(name, srcd, ci):
                t_ = pp.tile([128, DS], BF16, tag=f"{name}{ci}", name=f"{name}{ci}")
                nc.sync.dma_start(t_[:, :], srcd[128 * ci : 128 * (ci + 1), :])
                return t_

            # SP queue: xT even tiles interleaved with the wq tiles (the
            # first projection chain needs all of both), then wk, then wv
            wq_sb, wk_sb, wv_sb = [], [], []
            for ci in range(0, NCT, 2):
                t_ = pp.tile([128, T], BF16, tag=f"xt{ci}", name=f"xt{ci}")
                nc.sync.dma_start(t_[:, :], xT[128 * ci : 128 * (ci + 1), :])
                xT_sb[ci] = t_
                wq_sb.append(_wtile("wq", wq, ci))
                wq_sb.append(_wtile("wq", wq, ci + 1))
            for ci in range(NCT):
                wk_sb.append(_wtile("wk", wk, ci))
            for ci in range(NCT):
                wv_sb.append(_wtile("wv", wv, ci))

            # causal mask (needed from the first attention q-chunk on)
            nc.scalar.dma_start(mask_sb[:, :], maskd[:, :])

            # V tiles initialized with the ones column pattern
            vp_sb = []
            for h in range(HPC):
                t_ = pp.tile([128, NKT * 65], BF16, tag=f"vp{h}")
                nc.scalar.dma_start(t_[:, :], onesv[:, :])
                vp_sb.append(t_)

            # output-projection row-slice weights + bias/4
            wo_sb = []
            for hp in range(2):
                t_ = pwo.tile([128, C], BF16, tag=f"wo{hp}", name=f"wo{hp}")
                nc.scalar.dma_start(t_[:, :], wo2[128 * hp : 128 * (hp + 1), :])
                wo_sb.append(t_)
            bo_sb = pwo.tile([128, C], F32, tag="bo")
            nc.scalar.dma_start(bo_sb[:, :], bo4[:, :])

            # prime the Exp activation table before attention needs it
            dummy = pn.tile([1, 1], F32, tag="dummy")
            nc.scalar.activation(dummy[:, :], ones_t[0:1, 0:1], AF.Exp)

            qT_sb = [
                pp.tile([128, T], BF16, tag=f"qT{mt}", name=f"qT{mt}")
                for mt in range(2)
            ]
            kT_sb = [
                pp.tile([128, T], BF16, tag=f"kT{mt}", name=f"kT{mt}")
                for mt in range(2)
            ]

            # ---- interleaved per-stage emission: Q/K proj chunk, V proj
            # chunk, attention q-chunk, partial out-proj, ReduceScatter --
            for s in range(NQC):
                # Q^T / K^T projections for t-columns [512s, 512(s+1))
                for dst, wsb, bsb in ((qT_sb, wq_sb, bq_sb), (kT_sb, wk_sb, bk_sb)):
                    for mt in range(2):
                        ps = psA.tile([128, 512], F32, tag="sps")
                        for ci in range(NCT):
                            nc.tensor.matmul(
                                ps[:, :],
                                wsb[ci][:, 128 * mt : 128 * (mt + 1)],
                                xT_sb[ci][:, 512 * s : 512 * (s + 1)],
                                start=(ci == 0),
                                stop=(ci == NCT - 1),
                            )
                        nc.vector.tensor_scalar_add(
                            dst[mt][:, 512 * s : 512 * (s + 1)],
                            ps[:, :],
                            bsb[:, mt : mt + 1],
                        )

                # V projection for k-tiles [4s, 4s+4)
                for tt in range(4 * s, 4 * s + 4):
                    ps = psA.tile([128, DS], F32, tag="sps")
                    for ci in range(NCT):
                        nc.tensor.matmul(
                            ps[:, :],
                            xT_sb[ci][:, 128 * tt : 128 * (tt + 1)],
                            wv_sb[ci][:, :],
                            start=(ci == 0),
                            stop=(ci == NCT - 1),
                        )
                    for h in range(HPC):
                        nc.vector.tensor_copy(
                            vp_sb[h][:, 65 * tt : 65 * tt + 64],
                            ps[:, 64 * h : 64 * (h + 1)],
                        )

                # ---- causal attention for q-chunk s (S^T orientation) --
                # Head pairs share each k-tile step; PV accumulations trail
                # the S/exp front by two k-steps as same-bank pairs.
                qc = s
                nkt = 4 * qc + 4
                for hp in (0, 2):
                    ops = {}
                    for h in (hp, hp + 1):
                        ops[h] = psO.tile(
                            [65, 512], F32, tag=f"ops{h % 2}", name=f"op_q{qc}h{h}"
                        )
                    # Diagonal blocks (dm = kt - 4*qc >= 1) have their first
                    # 128*dm q-columns fully masked -- skip computing them.
                    # The PSUM accumulation group must start AND stop with
                    # full-width matmuls over the same zero regions, so for
                    # qc > 0 the PV of k-tile 4*qc-1 (non-diagonal, full
                    # width) is deferred to the end as the stop matmul;
                    # accumulation order is commutative. qc 0 has no
                    # deferral (all its tiles write full width).
                    def _c0(kt):
                        dm = kt - 4 * qc
                        return 128 * dm if (qc > 0 and dm >= 1) else 0

                    stop_kt = 4 * qc - 1 if qc > 0 else nkt - 1

                    def _pv(h, k2):
                        c0 = _c0(k2)
                        nc.tensor.matmul(
                            ops[h][:, c0:512],
                            vp_sb[h][:, 65 * k2 : 65 * k2 + 65],
                            pTs.pop((h, k2))[:, c0:512],
                            start=(k2 == 0),
                            stop=(k2 == stop_kt),
                        )

                    pTs = {}
                    for kt in range(nkt + 2):
                        if kt < nkt:
                            c0 = _c0(kt)
                            for h in (hp, hp + 1):
                                mt, pof = h // 2, 64 * (h % 2)
                                qs = qT_sb[mt][
                                    pof : pof + 64, 512 * qc + c0 : 512 * (qc + 1)
                                ]
                                st = psA.tile([128, 512], F32, tag="sps")
                                nc.tensor.matmul(
                                    st[:, c0:512],
                                    kT_sb[mt][pof : pof + 64, 128 * kt : 128 * (kt + 1)],
                                    qs,
                                    start=True,
                                    stop=True,
                                )
                                pT = pw.tile([128, 512], BF16, tag="pT")
                                nc.scalar.activation(
                                    pT[:, c0:512], st[:, c0:512], AF.Exp, scale=0.125
                                )
                                dm = kt - 4 * qc
                                if dm >= 0:  # diagonal block: causal mask
                                    nc.vector.tensor_tensor(
                                        pT[:, c0:512],
                                        pT[:, c0:512],
                                        mask_sb[:, 512 * dm + c0 : 512 * (dm + 1)],
                                        ALU.mult,
                                    )
                                pTs[(h, kt)] = pT
                        if kt % 2 == 1 and kt - 1 - 2 >= 0:
                            kv = kt - 1 - 2
                            for h in (hp, hp + 1):
                                for k2 in (kv, kv + 1):
                                    if k2 != stop_kt:
                                        _pv(h, k2)
                    for h in (hp, hp + 1):
                        ks = sorted(k for (hh, k) in pTs if hh == h)
                        for k2 in ks:
                            if k2 != stop_kt:
                                _pv(h, k2)
                        _pv(h, stop_kt)
                    for h in (hp, hp + 1):
                        _normalize(
                            nc, pn, psOut, ones_t, bv_sb, attn_sb, ops[h], h, qc,
                            f"op{h % 2}",
                        )

                # ---- partial output projection for q-chunk s -----------
                # partial[512s+128tt, 512cc] += attn^T @ Wo_rows (+ bo/GP)
                for tt in range(4):
                    for cc in range(2):
                        ps = psOut.tile(
                            [128, 512], F32, tag=f"op{(2 * tt + cc) % 2}",
                            name=f"out_ps{s}_{tt}_{cc}",
                        )
                        for hp in range(2):
                            nc.tensor.matmul(
                                ps[:, :],
                                attn_sb[(hp, s // 2)][
                                    :, 512 * (s % 2) + 128 * tt : 512 * (s % 2) + 128 * (tt + 1)
                                ],
                                wo_sb[hp][:, 512 * cc : 512 * (cc + 1)],
                                start=(hp == 0),
                                stop=(hp == 1),
                            )
                        osb = posb.tile([128, 512], BF16, tag="osb")
                        nc.vector.tensor_tensor(
                            osb[:, :], ps[:, :], bo_sb[:, 512 * cc : 512 * (cc + 1)],
                            ALU.add,
                        )
                        nc.sync.dma_start(
                            partial[
                                512 * s + 128 * tt : 512 * s + 128 * (tt + 1),
                                512 * cc : 512 * (cc + 1),
                            ],
                            osb[:, :],
                        )

                # ---- ReduceScatters: rows 0:1536 after qc2, the rest
                # after qc3. Collectives cannot write IO tensors, so they
                # land in rs_tmp; SBUF-hop DMAs on the gpsimd queue (which
                # just ran the collective, so the dependency is local)
                # forward each shard to the output. ----------------------
                def _forward(r0, r1):
                    for rt in range(r0, r1, 128):
                        fsb = pfw.tile([128, C], BF16, tag="fwd", name=f"fwd{rt}")
                        nc.gpsimd.dma_start(fsb[:, :], rs_tmp[rt : rt + 128, :])
                        nc.gpsimd.dma_start(rs_out[rt : rt + 128, :], fsb[:, :])

                if s == 2:
                    nc.gpsimd.collective_compute(
                        "ReduceScatter",
                        ALU.add,
                        replica_groups=[[0, 1, 2, 3], [4, 5, 6, 7]],
                        ins=[partial[0:RSA_ROWS, :]],
                        outs=[rs_tmp[0 : RSA_ROWS // GP, :]],
                    )
                    _forward(0, RSA_ROWS // GP)
                if s == 3:
                    nc.gpsimd.collective_compute(
                        "ReduceScatter",
                        ALU.add,
                        replica_groups=[[0, 1, 2, 3], [4, 5, 6, 7]],
                        ins=[partial[RSA_ROWS:T, :]],
                        outs=[rs_tmp[RSA_ROWS // GP : T // GP, :]],
                    )
                    _forward(RSA_ROWS // GP, T // GP)


def _build_program():
    nc = bacc.Bacc(
        "TRN2",
        target_bir_lowering=False,
        debug=False,
        num_devices=NCORES,
    )
    xT = nc.dram_tensor("xT", [C, T], BF16, kind="ExternalInput")
    wq = nc.dram_tensor("wq", [C, DS], BF16, kind="ExternalInput")
    wk = nc.dram_tensor("wk", [C, DS], BF16, kind="ExternalInput")
    wv = nc.dram_tensor("wv", [C, DS], BF16, kind="ExternalInput")
    wo2 = nc.dram_tensor("wo2", [DS, C], BF16, kind="ExternalInput")
    bq2 = nc.dram_tensor("bq2", [128, 2], F32, kind="ExternalInput")
    bk2 = nc.dram_tensor("bk2", [128, 2], F32, kind="ExternalInput")
    bv4 = nc.dram_tensor("bv4", [64, 4], F32, kind="ExternalInput")
    bo4 = nc.dram_tensor("bo4", [128, C], F32, kind="ExternalInput")
    maskd = nc.dram_tensor("maskd", [128, 4 * 512], BF16, kind="ExternalInput")
    onesd = nc.dram_tensor("onesd", [128, 64], F32, kind="ExternalInput")
    onesv = nc.dram_tensor("onesv", [128, NKT * 65], BF16, kind="ExternalInput")
    partial = nc.dram_tensor("partial", [T, C], BF16)
    rs_tmp = nc.dram_tensor("rs_tmp", [T // GP, C], BF16)
    rs_out = nc.dram_tensor("rs_out", [T // GP, C], BF16, kind="ExternalOutput")
    io = (xT, wq, wk, wv, wo2, bq2, bk2, bv4, bo4, maskd, onesd, onesv,
          partial, rs_tmp, rs_out)
    with tile.TileContext(nc) as tc:
        _emit(nc, tc, io)
    nc.compile()
    return nc


def _make_mask():
    # multiplicative causal mask blocks for the 4 diagonal positions:
    # 1 where k is visible (128*m + k_local <= q_local), 0 otherwise
    k = np.arange(128, dtype=np.int64)[:, None]
    q = np.arange(512, dtype=np.int64)[None, :]
    mask = np.zeros((128, 4 * 512), np.float32)
    for m in range(4):
        mask[:, 512 * m : 512 * (m + 1)] = (128 * m + k <= q).astype(np.float32)
    return mask.astype(NPBF)


def _make_in_maps(x, Wq, bq, Wk, bk, Wv, bv, Wo, bo):
    mask = _make_mask()
    in_maps = []
    for c in range(NCORES):
        b, g = c // GP, c % GP
        hs = slice(DS * g, DS * (g + 1))
        in_maps.append(
            {
                "xT": np.ascontiguousarray(x[b].T).astype(NPBF),
                "wq": np.ascontiguousarray(Wq[:, hs]).astype(NPBF),
                "wk": np.ascontiguousarray(Wk[:, hs]).astype(NPBF),
                "wv": np.ascontiguousarray(Wv[:, hs]).astype(NPBF),
                "wo2": np.ascontiguousarray(Wo[hs, :]).astype(NPBF),
                "bq2": np.ascontiguousarray(bq[hs].reshape(2, 128).T),
                "bk2": np.ascontiguousarray(bk[hs].reshape(2, 128).T),
                "bv4": np.ascontiguousarray(bv[hs].reshape(4, 64).T),
                "bo4": np.tile((bo / GP)[None, :], (128, 1)).astype(np.float32),
                "maskd": mask,
                "onesd": np.ones((128, 64), np.float32),
                "onesv": np.ones((128, NKT * 65), NPBF),
            }
        )
    return in_maps


def kernel(x, Wq, bq, Wk, bk, Wv, bv, Wo, bo, _trace=False, _trace_cores=None):
    global _PROG, LAST_RESULTS
    x = np.asarray(x, np.float32)
    Wq, bq = np.asarray(Wq, np.float32), np.asarray(bq, np.float32)
    Wk, bk = np.asarray(Wk, np.float32), np.asarray(bk, np.float32)
    Wv, bv = np.asarray(Wv, np.float32), np.asarray(bv, np.float32)
    Wo, bo = np.asarray(Wo, np.float32), np.asarray(bo, np.float32)

    if _PROG is None:
        _PROG = _build_program()
    nc = _PROG

    in_maps = _make_in_maps(x, Wq, bq, Wk, bk, Wv, bv, Wo, bo)

    kw = {}
    if _trace:
        kw["trace"] = True
        if _trace_cores is not None:
            kw["trace_cores"] = _trace_cores
    res = bass_utils.run_bass_kernel_spmd(nc, in_maps, list(range(NCORES)), **kw)
    LAST_RESULTS = res

    out = np.empty((B, T, C), np.float32)
    na = RSA_ROWS // GP  # 384 rows from the first ReduceScatter
    for c in range(NCORES):
        b, g = c // GP, c % GP
        rs = np.asarray(res.results[c]["rs_out"]).astype(np.float32)
        out[b, na * g : na * (g + 1), :] = rs[0:na]
        out[b, RSA_ROWS + 128 * g : RSA_ROWS + 128 * (g + 1), :] = rs[na:]
    return out



# revision 6
# speedup vs baseline: 1.2786x; 1.2786x over previous
"""Causal multi-head attention (B=2, T=2048, C=1024, H=16) on 8 TRN2 cores.

Sharding: data-parallel over batch (2 groups of 4 cores), tensor-parallel
over heads within a group (4 heads / core). Each core:
  1. computes Q^T, K^T (layout [d, t]) and V (layout [t, d]) for its heads
     from x[b]^T (host-transposed) and its W column slices; bv is folded
     into the V tiles at the PSUM->SBUF copy,
  2. runs causal attention in the S^T = K @ Q^T orientation (softmax sums
     come for free from a ones-column appended to V; softmax max-subtraction
     is skipped -- scores are O(1) here so exp is safe). Head pairs share
     one [128, 2, 512] PSUM tile so exp runs as a single wide ACTIVATE; all
     diagonal blocks are column-trimmed and masked with one 128x128
     triangle tile,
  3. computes a PARTIAL output projection over its 256 attention dims
     (contraction = its head slice of Wo rows) for the full [T, C] output,
  4. ReduceScatters the partials (op=add) within its 4-core batch group in
     SIX pipelined pieces (last piece only 128 rows so the network tail is
     short), landing each shard in rs_tmp; gpsimd-queue DMAs forward the
     shards to the rs_out ExternalOutput. Host reassembles the pieces.

Matmul operands are bf16 (fp32 PSUM accumulation); softmax normalization
uses reciprocal_approx_fast (DVE custom op, ~18-bit) broadcast via a ones
matmul.
"""

import os
import sys

import numpy as np
import ml_dtypes

for _p in ("/opt/trn_rl_repo",):
    if os.path.isdir(_p) and _p not in sys.path:
        sys.path.insert(0, _p)

import concourse.bacc as bacc
import concourse.mybir as mybir
import concourse.tile as tile
from concourse import bass_utils

B, T, C, H, D = 2, 2048, 1024, 16, 64
NCORES = 8
GP = 4              # cores per batch group
HPC = H // GP       # heads per core = 4
DS = HPC * D        # per-core head-dim slice = 256
NCT = C // 128      # c-tiles = 8
NQC = T // 512      # q-chunks = 4
NKT = T // 128      # k-tiles = 16

# ReduceScatter pieces: (row0, row1) of `partial`; fired as soon as the
# out-projection rows are written. Last piece kept small (128 rows) to
# minimize the exposed network tail.
RS_PIECES = [(0, 512), (512, 1024), (1024, 1536), (1536, 1792), (1792, 1920), (1920, 2048)]

F32 = mybir.dt.float32
F32R = mybir.dt.float32r
BF16 = mybir.dt.bfloat16
AF = mybir.ActivationFunctionType
ALU = mybir.AluOpType
NPBF = ml_dtypes.bfloat16

_PROG = None
LAST_RESULTS = None  # BassKernelResults of the most recent run (for test.py)


def _r(ap):
    return ap.bitcast(F32R)


def _normalize(nc, pn, psOut, ones_row, attn_sb, op_ps, h, qc, tag):
    """Divide O' rows 0..63 by the row-sums (row 64), write attn slice."""
    sums_sb = pn.tile([1, 512], F32, tag="sums")
    nc.vector.tensor_copy(sums_sb[:, :], op_ps[64:65, :])
    recip = pn.tile([1, 512], F32, tag="recip")
    nc.vector.reciprocal_approx_fast(recip[:, :], sums_sb[:, :])
    recip_bf = pn.tile([1, 512], BF16, tag="recipbf")
    nc.vector.tensor_copy(recip_bf[:, :], recip[:, :])
    bc_ps = psOut.tile([64, 512], F32, tag=tag, name=f"bc_{h}_{qc}")
    nc.tensor.matmul(
        bc_ps[:, :],
        ones_row[:, 0:64],
        recip_bf[:, :],
        start=True,
        stop=True,
    )
    bc_sb = pn.tile([64, 512], BF16, tag="bc")
    nc.vector.tensor_copy(bc_sb[:, :], bc_ps[:, :])
    po = 64 * (h % 2)
    aslc = attn_sb[(h // 2, qc // 2)][po : po + 64, 512 * (qc % 2) : 512 * (qc % 2 + 1)]
    nc.vector.tensor_tensor(aslc, op_ps[0:64, :], bc_sb[:, :], ALU.mult)


def _emit(nc, tc, io):
    (xT, wq, wk, wv, wo2, bq2, bk2, bvr, bor, trid,
     partial, rs_tmp, rs_out) = io

    with (
        tc.tile_pool(name="outer", bufs=1) as po,
    ):
        # ---- small persistent tiles (scalar-engine DMA queue) ----------
        tri_sb = po.tile([128, 128], BF16, tag="tri")
        bq_sb = po.tile([128, 2], F32, tag="bq")
        nc.scalar.dma_start(bq_sb[:, :], bq2[:, :])
        bk_sb = po.tile([128, 2], F32, tag="bk")
        nc.scalar.dma_start(bk_sb[:, :], bk2[:, :])
        bv_row = po.tile([1, DS], BF16, tag="bvr")
        nc.scalar.dma_start(bv_row[:, :], bvr[:, :])
        bo_row = po.tile([1, C], BF16, tag="bor")
        nc.scalar.dma_start(bo_row[:, :], bor[:, :])
        nc.scalar.dma_start(tri_sb[:, :], trid[:, :])
        ones_row = po.tile([1, 128], BF16, tag="onesrow")
        nc.vector.memset(ones_row[:, :], 1.0)

        attn_sb = {}
        for hp in range(2):
            for hf in (0, 1):
                attn_sb[(hp, hf)] = po.tile(
                    [128, T // 2], BF16, tag=f"attn{hp}_{hf}", name=f"attn{hp}_{hf}"
                )

        with (
            tc.tile_pool(name="proj", bufs=1) as pp,
            tc.tile_pool(name="work", bufs=6) as pw,
            tc.tile_pool(name="nrm", bufs=2) as pn,
            tc.tile_pool(name="wos", bufs=1) as pwo,
            tc.tile_pool(name="osb", bufs=4) as posb,
            tc.tile_pool(name="fwd", bufs=2) as pfw,
            tc.tile_pool(name="psA", bufs=2, space="PSUM") as psA,
            tc.tile_pool(name="psO", bufs=1, space="PSUM") as psO,
            tc.tile_pool(name="psOut", bufs=1, space="PSUM") as psOut,
        ):
            # ---- input loads: xT split across the two HWDGE queues (SP
            # even tiles, Activation odd tiles); W triples ride SP after
            # their xT tile ------------------------------------------------
            xT_sb = [None] * NCT
            for ci in range(1, NCT, 2):
                t_ = pp.tile([128, T], BF16, tag=f"xt{ci}", name=f"xt{ci}")
                nc.scalar.dma_start(t_[:, :], xT[128 * ci : 128 * (ci + 1), :])
                xT_sb[ci] = t_

            def _wtile(name, srcd, ci):
                t_ = pp.tile([128, DS], BF16, tag=f"{name}{ci}", name=f"{name}{ci}")
                nc.sync.dma_start(t_[:, :], srcd[128 * ci : 128 * (ci + 1), :])
                return t_

            # SP queue: xT even tiles interleaved with the wq tiles (the
            # first projection chain needs all of both), then wk, then wv
            wq_sb, wk_sb, wv_sb = [], [], []
            for ci in range(0, NCT, 2):
                t_ = pp.tile([128, T], BF16, tag=f"xt{ci}", name=f"xt{ci}")
                nc.sync.dma_start(t_[:, :], xT[128 * ci : 128 * (ci + 1), :])
                xT_sb[ci] = t_
                wq_sb.append(_wtile("wq", wq, ci))
                wq_sb.append(_wtile("wq", wq, ci + 1))
            for ci in range(NCT):
                wk_sb.append(_wtile("wk", wk, ci))
            for ci in range(NCT):
                wv_sb.append(_wtile("wv", wv, ci))

            # V tiles: only the per-k-tile ones column (stride 65) needs
            # initialization; the rest is overwritten by the V projection.
            vp_sb = []
            for h in range(HPC):
                t_ = pp.tile([128, NKT, 65], BF16, tag=f"vp{h}", name=f"vp{h}")
                nc.vector.memset(t_[:, :, 64:65], 1.0)
                vp_sb.append(t_)

            # bv broadcast tile [128, DS] via ones-column matmul
            bvb_ps = psOut.tile([128, DS], F32, tag="op0", name="bvb_ps")
            nc.tensor.matmul(bvb_ps[:, :], ones_row[:, :], bv_row[:, :],
                             start=True, stop=True)
            bvb_sb = po.tile([128, DS], BF16, tag="bvb")
            nc.vector.tensor_copy(bvb_sb[:, :], bvb_ps[:, :])

            # bo broadcast tile [128, C] (bo/GP is pre-divided on host)
            bo_sb = po.tile([128, C], BF16, tag="bo")
            for cc in range(2):
                bob_ps = psOut.tile([128, 512], F32, tag="op1", name=f"bob{cc}")
                nc.tensor.matmul(bob_ps[:, :], ones_row[:, :],
                                 bo_row[:, 512 * cc : 512 * (cc + 1)],
                                 start=True, stop=True)
                nc.vector.tensor_copy(
                    bo_sb[:, 512 * cc : 512 * (cc + 1)], bob_ps[:, :]
                )

            # output-projection row-slice weights
            wo_sb = []
            for hp in range(2):
                t_ = pwo.tile([128, C], BF16, tag=f"wo{hp}", name=f"wo{hp}")
                nc.scalar.dma_start(t_[:, :], wo2[128 * hp : 128 * (hp + 1), :])
                wo_sb.append(t_)

            # prime the Exp activation table before attention needs it
            dummy = pn.tile([1, 1], F32, tag="dummy")
            nc.scalar.activation(dummy[:, :], bq_sb[0:1, 0:1], AF.Exp)

            qT_sb = [
                pp.tile([128, T], BF16, tag=f"qT{mt}", name=f"qT{mt}")
                for mt in range(2)
            ]
            kT_sb = [
                pp.tile([128, T], BF16, tag=f"kT{mt}", name=f"kT{mt}")
                for mt in range(2)
            ]

            # ---- ReduceScatter plumbing --------------------------------
            rs_done = [False] * len(RS_PIECES)

            def _fire_rs(rows_ready):
                for pi, (r0, r1) in enumerate(RS_PIECES):
                    if rs_done[pi] or r1 > rows_ready:
                        continue
                    rs_done[pi] = True
                    nc.gpsimd.collective_compute(
                        "ReduceScatter",
                        ALU.add,
                        replica_groups=[[0, 1, 2, 3], [4, 5, 6, 7]],
                        ins=[partial[r0:r1, :]],
                        outs=[rs_tmp[r0 // GP : r1 // GP, :]],
                    )
                    # Forward the shard to the ExternalOutput (collectives
                    # cannot write IO tensors). The gpsimd queue just ran
                    # the collective so the dependency is queue-local.
                    for rt in range(r0 // GP, r1 // GP, 128):
                        re = min(rt + 128, r1 // GP)
                        fsb = pfw.tile([128, C], BF16, tag="fwd", name=f"fwd{rt}")
                        nc.gpsimd.dma_start(fsb[0 : re - rt, :], rs_tmp[rt:re, :])
                        nc.gpsimd.dma_start(rs_out[rt:re, :], fsb[0 : re - rt, :])

            # ---- interleaved per-stage emission: Q/K proj chunk, V proj
            # chunk, attention q-chunk, partial out-proj, ReduceScatter --
            for s in range(NQC):
                # Q^T / K^T projections for t-columns [512s, 512(s+1))
                for dst, wsb, bsb in ((qT_sb, wq_sb, bq_sb), (kT_sb, wk_sb, bk_sb)):
                    ps = psA.tile([128, 2, 512], F32, tag="sps", name=f"pj{s}")
                    for mt in range(2):
                        for ci in range(NCT):
                            nc.tensor.matmul(
                                ps[:, mt, :],
                                wsb[ci][:, 128 * mt : 128 * (mt + 1)],
                                xT_sb[ci][:, 512 * s : 512 * (s + 1)],
                                start=(ci == 0),
                                stop=(ci == NCT - 1),
                            )
                    for mt in range(2):
                        nc.vector.tensor_scalar_add(
                            dst[mt][:, 512 * s : 512 * (s + 1)],
                            ps[:, mt, :],
                            bsb[:, mt : mt + 1],
                        )

                # V projection for k-tiles [4s, 4s+4); vp gets +bv folded in
                for half in range(2):
                    ps = psA.tile([128, 2, 512], F32, tag="sps", name=f"vj{s}{half}")
                    for sub in range(2):
                        tt = 4 * s + 2 * half + sub
                        for ci in range(NCT):
                            nc.tensor.matmul(
                                ps[:, sub, 0:DS],
                                xT_sb[ci][:, 128 * tt : 128 * (tt + 1)],
                                wv_sb[ci][:, :],
                                start=(ci == 0),
                                stop=(ci == NCT - 1),
                            )
                    for sub in range(2):
                        tt = 4 * s + 2 * half + sub
                        for h in range(HPC):
                            nc.vector.tensor_tensor(
                                vp_sb[h][:, tt, 0:64],
                                ps[:, sub, 64 * h : 64 * (h + 1)],
                                bvb_sb[:, 64 * h : 64 * (h + 1)],
                                ALU.add,
                            )

                # ---- causal attention for q-chunk s (S^T orientation) --
                # Head pairs share one [128, 2, 512] S/exp tile per k-tile
                # (the two S matmuls row-pack into the PE as base-partition
                # 0/64 tiles); PV accumulations trail the S/exp front by
                # two k-steps.  Diagonal blocks (dm = kt - 4*qc >= 0) have
                # their first 128*dm q-columns fully masked -- skip them.
                qc = s
                nkt = 4 * qc + 4

                def _c0(kt):
                    return max(0, 128 * (kt - 4 * qc))

                for hp in (0, 2):
                    ops = {}
                    for h in (hp, hp + 1):
                        ops[h] = psO.tile(
                            [65, 512], F32, tag=f"ops{h % 2}", name=f"op_q{qc}h{h}"
                        )
                    # The PSUM accumulation group starts with the (full
                    # width) kt=0 matmul.  For qc > 0 the PV of k-tile
                    # 4*qc-1 (non-diagonal, full width) is deferred to the
                    # end as the stop matmul; for qc == 0 the last trimmed
                    # PV carries the stop flag (accumulation order is
                    # commutative; has_written covers the trimmed columns).
                    stop_kt = 4 * qc - 1 if qc > 0 else nkt - 1

                    def _pv(h, k2):
                        c0 = _c0(k2)
                        nc.tensor.matmul(
                            ops[h][:, c0:512],
                            vp_sb[h][:, k2, :],
                            pTs.pop((h, k2))[:, c0:512],
                            start=(k2 == 0),
                            stop=(k2 == stop_kt),
                            skip_group_check=(qc == 0),
                        )

                    pTs = {}
                    for kt in range(nkt + 2):
                        if kt < nkt:
                            c0 = _c0(kt)
                            st2 = psA.tile([128, 2, 512], F32, tag="sps",
                                           name=f"st{qc}_{hp}_{kt}")
                            for h in (hp, hp + 1):
                                mt, pof = h // 2, 64 * (h % 2)
                                nc.tensor.matmul(
                                    st2[:, h % 2, c0:512],
                                    kT_sb[mt][pof : pof + 64, 128 * kt : 128 * (kt + 1)],
                                    qT_sb[mt][pof : pof + 64, 512 * qc + c0 : 512 * (qc + 1)],
                                    start=True,
                                    stop=True,
                                )
                            pT2 = pw.tile([128, 2, 512], BF16, tag="pT",
                                          name=f"pT{qc}_{hp}_{kt}")
                            nc.scalar.activation(
                                pT2[:, :, c0:512], st2[:, :, c0:512], AF.Exp,
                                scale=0.125,
                            )
                            dm = kt - 4 * qc
                            if dm >= 0:  # diagonal block: mask the 128-wide
                                for h2 in range(2):  # triangle region only
                                    nc.vector.tensor_tensor(
                                        pT2[:, h2, c0 : c0 + 128],
                                        pT2[:, h2, c0 : c0 + 128],
                                        tri_sb[:, :],
                                        ALU.mult,
                                    )
                            pTs[(hp, kt)] = pT2[:, 0, :]
                            pTs[(hp + 1, kt)] = pT2[:, 1, :]
                        if kt % 2 == 1 and kt - 1 - 2 >= 0:
                            kv = kt - 1 - 2
                            for h in (hp, hp + 1):
                                for k2 in (kv, kv + 1):
                                    if k2 != stop_kt:
                                        _pv(h, k2)
                    for h in (hp, hp + 1):
                        ks = sorted(k for (hh, k) in pTs if hh == h)
                        for k2 in ks:
                            if k2 != stop_kt:
                                _pv(h, k2)
                        _pv(h, stop_kt)
                    for h in (hp, hp + 1):
                        _normalize(
                            nc, pn, psOut, ones_row, attn_sb, ops[h], h, qc,
                            f"op{h % 2}",
                        )

                # ---- partial output projection for q-chunk s -----------
                # partial[512s+128tt, 512cc] += attn^T @ Wo_rows (+ bo/GP)
                for tt in range(4):
                    for cc in range(2):
                        ps = psOut.tile(
                            [128, 512], F32, tag=f"op{(2 * tt + cc) % 2}",
                            name=f"out_ps{s}_{tt}_{cc}",
                        )
                        for hp in range(2):
                            nc.tensor.matmul(
                                ps[:, :],
                                attn_sb[(hp, s // 2)][
                                    :, 512 * (s % 2) + 128 * tt : 512 * (s % 2) + 128 * (tt + 1)
                                ],
                                wo_sb[hp][:, 512 * cc : 512 * (cc + 1)],
                                start=(hp == 0),
                                stop=(hp == 1),
                            )
                        osb = posb.tile([128, 512], BF16, tag="osb")
                        nc.vector.tensor_tensor(
                            osb[:, :], ps[:, :], bo_sb[:, 512 * cc : 512 * (cc + 1)],
                            ALU.add,
                        )
                        nc.sync.dma_start(
                            partial[
                                512 * s + 128 * tt : 512 * s + 128 * (tt + 1),
                                512 * cc : 512 * (cc + 1),
                            ],
                            osb[:, :],
                        )
                    _fire_rs(512 * s + 128 * (tt + 1))


def _build_program():
    nc = bacc.Bacc(
        "TRN2",
        target_bir_lowering=False,
        debug=False,
        num_devices=NCORES,
    )
    xT = nc.dram_tensor("xT", [C, T], BF16, kind="ExternalInput")
    wq = nc.dram_tensor("wq", [C, DS], BF16, kind="ExternalInput")
    wk = nc.dram_tensor("wk", [C, DS], BF16, kind="ExternalInput")
    wv = nc.dram_tensor("wv", [C, DS], BF16, kind="ExternalInput")
    wo2 = nc.dram_tensor("wo2", [DS, C], BF16, kind="ExternalInput")
    bq2 = nc.dram_tensor("bq2", [128, 2], F32, kind="ExternalInput")
    bk2 = nc.dram_tensor("bk2", [128, 2], F32, kind="ExternalInput")
    bvr = nc.dram_tensor("bvr", [1, DS], BF16, kind="ExternalInput")
    bor = nc.dram_tensor("bor", [1, C], BF16, kind="ExternalInput")
    trid = nc.dram_tensor("trid", [128, 128], BF16, kind="ExternalInput")
    partial = nc.dram_tensor("partial", [T, C], BF16)
    rs_tmp = nc.dram_tensor("rs_tmp", [T // GP, C], BF16)
    rs_out = nc.dram_tensor("rs_out", [T // GP, C], BF16, kind="ExternalOutput")
    io = (xT, wq, wk, wv, wo2, bq2, bk2, bvr, bor, trid,
          partial, rs_tmp, rs_out)
    with tile.TileContext(nc) as tc:
        _emit(nc, tc, io)
    nc.compile()
    return nc


def _make_in_maps(x, Wq, bq, Wk, bk, Wv, bv, Wo, bo):
    # multiplicative causal triangle for the in-diagonal 128x128 region:
    # 1 where k_local <= q_local, 0 otherwise
    k = np.arange(128, dtype=np.int64)[:, None]
    q = np.arange(128, dtype=np.int64)[None, :]
    tri = (k <= q).astype(np.float32).astype(NPBF)
    in_maps = []
    for c in range(NCORES):
        b, g = c // GP, c % GP
        hs = slice(DS * g, DS * (g + 1))
        in_maps.append(
            {
                "xT": np.ascontiguousarray(x[b].T).astype(NPBF),
                "wq": np.ascontiguousarray(Wq[:, hs]).astype(NPBF),
                "wk": np.ascontiguousarray(Wk[:, hs]).astype(NPBF),
                "wv": np.ascontiguousarray(Wv[:, hs]).astype(NPBF),
                "wo2": np.ascontiguousarray(Wo[hs, :]).astype(NPBF),
                "bq2": np.ascontiguousarray(bq[hs].reshape(2, 128).T),
                "bk2": np.ascontiguousarray(bk[hs].reshape(2, 128).T),
                "bvr": bv[hs].reshape(1, DS).astype(NPBF),
                "bor": (bo / GP).reshape(1, C).astype(NPBF),
                "trid": tri,
            }
        )
    return in_maps


def kernel(x, Wq, bq, Wk, bk, Wv, bv, Wo, bo, _trace=False, _trace_cores=None):
    global _PROG, LAST_RESULTS
    x = np.asarray(x, np.float32)
    Wq, bq = np.asarray(Wq, np.float32), np.asarray(bq, np.float32)
    Wk, bk = np.asarray(Wk, np.float32), np.asarray(bk, np.float32)
    Wv, bv = np.asarray(Wv, np.float32), np.asarray(bv, np.float32)
    Wo, bo = np.asarray(Wo, np.float32), np.asarray(bo, np.float32)

    if _PROG is None:
        _PROG = _build_program()
    nc = _PROG

    in_maps = _make_in_maps(x, Wq, bq, Wk, bk, Wv, bv, Wo, bo)

    kw = {}
    if _trace:
        kw["trace"] = True
        if _trace_cores is not None:
            kw["trace_cores"] = _trace_cores
    res = bass_utils.run_bass_kernel_spmd(nc, in_maps, list(range(NCORES)), **kw)
    LAST_RESULTS = res

    out = np.empty((B, T, C), np.float32)
    for c in range(NCORES):
        b, g = c // GP, c % GP
        rs = np.asarray(res.results[c]["rs_out"]).astype(np.float32)
        for r0, r1 in RS_PIECES:
            L = (r1 - r0) // GP
            out[b, r0 + L * g : r0 + L * (g + 1), :] = rs[r0 // GP : r0 // GP + L]
    return out
